# revision 73
# baseline (speedup 1.0000x reference)
"""Trainium2 Bass kernel for nn_DHSMoERBFDetector.

Reference math (B=8192, D=768, NC=4, R=128, E=20, H=1024):
    rbf[c,b,r] = exp(coeff[c] * (feats[c,b] - offset[c,r])^2)
    x = [emb | rbf-features]                      # [B, 1280]
    h_e = relu(x @ W1[e] + b1[e]); pred_e = h_e @ W2[e] + b2[e]
    out = concat_e(pred_e)[inv]  with inv = argsort(argsort(component_idx))

Key fact: inv has values < B, so only expert 0's predictions are ever
selected.  The output is exactly  (relu(x @ W1[0] + b1[0]) @ W2[0] + b2[0])[inv].

Strategy: data-parallel over batch, 1024 rows per core on 8 cores.
Each core computes x^T (K-major: contraction dim on partitions) in SBUF
and runs the expert-0 MLP (K=1280 in 10 chunks of 128; H as 8 chunks of
M=128; batch as N=512 moving operand) accumulating in PSUM.  The inverse
permutation is an int gather of 8192 floats, done on host.

Default mode "opt" (46.3us cost model vs the 136.1us 3-pass baseline;
HW rel err ~4.2e-3 vs the 2e-2 gate):
  - single-pass bf16 matmul1 (1 PE cycle/row; ~3e-3 err, 1/3 the matmuls
    of the residual-split modes and half the DMA bytes),
  - rbf K-chunks reordered FIRST (computed on-device while emb/W stream),
  - (feat-off) outer difference via a K=2 fp16 matmul [ones;-off]^T@[f;1],
    then exp(coef*d^2) = (sqrt(pi)/2)*erf'(sqrt(-coef)*d) as a SINGLE
    Derivative_Erf activation with a compile-time-immediate scale
    (rbf_coeff is baked into the program; _NC_CACHE is keyed on it; the
    sqrt(pi)/2 is folded into W1's rbf rows on host),
  - |w2| folded into W1/b1 columns on host + H sorted by sign(w2): the
    H->1 dot collapses to DVE chunk-adds + a few 1-cycle/row matmuls
    (ones / -ones / per-chunk sign columns for the final tile),
  - host-packed DMAs: ~13 large pieces, tiny blobs on SP's queue, big W/x
    pieces on Pool's (gpsimd issue is ~60ns vs SP/Act's ~1.2-2.3us; DMA
    transfers SERIALIZE at ~0.39ns/free-byte, round-robin across queues,
    and consumers pay a 900ns semaphore after each transfer), W1 split by
    (k-range, m-half) so only m0-3 halves are needed early,
  - a dummy activation at t~0 (pulls the 1.3us act-table load off the
    critical path) and a dummy matmul chain that starts the PE p-state
    ramp early (any PE idle resets the 2.4GHz ramp; gaps cost double).

Legacy modes via KERNEL_MODE: bf16x3/bf16x3b, fp32, f32r1, f32r3, f32rh
(f32rh crashed HW: mixed-dtype PSUM accumulation groups suspected).
"""

import os

import numpy as np

import concourse.bacc as bacc
import concourse.bass as bass
import concourse.mybir as mybir
import concourse.tile as tile
from concourse.bass_utils import run_bass_kernel_spmd

FP32 = mybir.dt.float32
F32R = mybir.dt.float32r
BF16 = mybir.dt.bfloat16
FP16 = mybir.dt.float16
AF = mybir.ActivationFunctionType

B, D, NCOL, R, E, H = 8192, 768, 4, 128, 20, 1024
KTOT = D + NCOL * R          # 1280 contraction dim
NCORES = 8
BL = B // NCORES             # 1024 batch rows per core
KC = KTOT // 128             # 10 k-chunks (0-5 emb, 6-9 rbf)
KC_EMB = D // 128            # 6
HC = H // 128                # 8 hidden chunks
NT = BL // 512               # 2 batch tiles of N=512


def _consts(nc, consts, dram_map):
    sb = {}
    sb["feats"] = consts.tile([1, NCOL * BL], FP32, tag="feats", name="feats_sb")
    sb["noff"] = consts.tile([R, NCOL], FP32, tag="noff", name="noff_sb")
    sb["coef"] = consts.tile([R, NCOL], FP32, tag="coef", name="coef_sb")
    sb["b1"] = consts.tile([128, HC], FP32, tag="b1", name="b1_sb")
    sb["b2"] = consts.tile([1, 1], FP32, tag="b2", name="b2_sb")
    sb["ones"] = consts.tile([1, 128], FP32, tag="ones", name="ones_sb")
    nc.vector.memset(sb["ones"], 1.0)
    for key, src in dram_map.items():
        nc.sync.dma_start(sb[key], src[:, :])
    return sb


def _rbf_psum(nc, pp, sb, c, n):
    """Broadcast feats[c] across partitions into a PSUM tile via K=1 matmul."""
    bc = pp.tile([128, 512], FP32, tag="ps", name=f"bc_{c}_{n}")
    nc.tensor.matmul(
        bc, lhsT=sb["ones"][:, :],
        rhs=sb["feats"][:, c * BL + n * 512 : c * BL + (n + 1) * 512],
        start=True, stop=True,
    )
    return bc


def _build_f32r(nc, tc, dram, pools, three_pass: bool):
    """f32r matmul pipeline; three_pass adds the two residual terms.

    three_pass keeps SBUF under budget by streaming feats tiles, keeping h
    chunks per-n-tile only ([128,512] per m, reused across n), and using a
    plain fp32 second matmul (no h split needed).
    """
    big, consts, tmp, outp, pp = pools
    d = dram
    sb = {}
    sb["noff"] = consts.tile([R, NCOL], FP32, tag="noff", name="noff_sb")
    sb["coef"] = consts.tile([R, NCOL], FP32, tag="coef", name="coef_sb")
    sb["b1"] = consts.tile([128, HC], FP32, tag="b1", name="b1_sb")
    sb["b2"] = consts.tile([1, 1], FP32, tag="b2", name="b2_sb")
    sb["ones"] = consts.tile([1, 128], FP32, tag="ones", name="ones_sb")
    nc.vector.memset(sb["ones"], 1.0)
    for key, src in [("noff", d["noff"]), ("coef", d["coef"]),
                     ("b1", d["b1c"]), ("b2", d["b2c"])]:
        nc.sync.dma_start(sb[key], src[:, :])
    w2r_sb = consts.tile([128, HC], F32R, tag="w2r", name="w2r_sb")
    nc.sync.dma_start(w2r_sb, d["w2r"][:, :])
    w2f_sb = None
    if three_pass:
        w2f_sb = consts.tile([128, HC], FP32, tag="w2f", name="w2f_sb")
        nc.sync.dma_start(w2f_sb, d["w2c"][:, :])

    xr = [big.tile([128, BL], F32R, tag=f"xr{k}", name=f"xr{k}")
          for k in range(KC)]
    wr = [big.tile([128, H], F32R, tag=f"wr{k}", name=f"wr{k}")
          for k in range(KC)]
    if three_pass:
        xl = [big.tile([128, BL], F32R, tag=f"xl{k}", name=f"xl{k}")
              for k in range(KC)]
        wl = [big.tile([128, H], F32R, tag=f"wl{k}", name=f"wl{k}")
              for k in range(KC)]

    # RBF features first: their small feats DMAs must not queue behind the
    # big weight DMAs (the in-order PE's first instruction waits on them).
    for c in range(NCOL):
        for n in range(NT):
            bsl = slice(n * 512, (n + 1) * 512)
            kk = KC_EMB + c
            fe = tmp.tile([1, 512], FP32, tag="fe")
            nc.sync.dma_start(
                fe, d["feats"][:, c * BL + n * 512 : c * BL + (n + 1) * 512])
            bc = pp.tile([128, 512], FP32, tag="ps", name=f"bc_{c}_{n}")
            nc.tensor.matmul(bc, lhsT=sb["ones"][:, :], rhs=fe,
                             start=True, stop=True)
            d2 = tmp.tile([128, 512], FP32, tag="d2")
            nc.scalar.activation(d2, bc, AF.Square,
                                 bias=sb["noff"][:, c : c + 1], scale=1.0)
            if three_pass:
                rb = tmp.tile([128, 512], FP32, tag="rb")
                nc.scalar.activation(rb, d2, AF.Exp,
                                     scale=sb["coef"][:, c : c + 1])
                nc.vector.tensor_copy(xr[kk][:, bsl], rb)   # round to f32r
                nc.vector.tensor_tensor(
                    xl[kk][:, bsl], rb, xr[kk][:, bsl].bitcast(FP32),
                    mybir.AluOpType.subtract,
                )                                           # residual, rounded
            else:
                nc.scalar.activation(xr[kk][:, bsl], d2, AF.Exp,
                                     scale=sb["coef"][:, c : c + 1])

    for k in range(KC):
        ksl = slice(k * 128, (k + 1) * 128)
        nc.sync.dma_start(wr[k][:, :], d["w1r"][ksl, :])
        if three_pass:
            nc.sync.dma_start(wl[k][:, :], d["w1l"][ksl, :])
        if k < KC_EMB:
            nc.sync.dma_start(xr[k][:, :], d["ehr"][ksl, :])
            if three_pass:
                nc.sync.dma_start(xl[k][:, :], d["ehl"][ksl, :])

    # h^T = relu(W1^T x + b1); f32r1 writes relu straight to f32r tiles
    h_dt = FP32 if three_pass else F32R
    h_len = 512 if three_pass else BL
    hs = [big.tile([128, h_len], h_dt, tag=f"h{m}", name=f"h{m}")
          for m in range(HC)]
    for n in range(NT):
        bsl = slice(n * 512, (n + 1) * 512)
        hsl = slice(0, 512) if three_pass else bsl
        # pred accumulates per group so the last group's relu drain overlaps
        # with the earlier groups' pred matmuls
        p2 = pp.tile([1, 512], FP32, tag="ps", name=f"p2_{n}")
        w2 = w2f_sb if three_pass else w2r_sb
        for g in range(2):
            ms = range(4 * g, 4 * g + 4)
            ps = {m: pp.tile([128, 512], FP32, tag="ps", name=f"ps_{n}_{g}_{m}")
                  for m in ms}
            for k in range(KC):
                for m in ms:
                    msl = slice(m * 128, (m + 1) * 128)
                    first, last = k == 0, k == KC - 1
                    if three_pass:
                        nc.tensor.matmul(ps[m], lhsT=wr[k][:, msl],
                                         rhs=xr[k][:, bsl],
                                         start=first, stop=False)
                        nc.tensor.matmul(ps[m], lhsT=wr[k][:, msl],
                                         rhs=xl[k][:, bsl],
                                         start=False, stop=False)
                        nc.tensor.matmul(ps[m], lhsT=wl[k][:, msl],
                                         rhs=xr[k][:, bsl],
                                         start=False, stop=last)
                    else:
                        nc.tensor.matmul(ps[m], lhsT=wr[k][:, msl],
                                         rhs=xr[k][:, bsl],
                                         start=first, stop=last)
            for m in ms:
                nc.scalar.activation(hs[m][:, hsl], ps[m], AF.Relu,
                                     bias=sb["b1"][:, m : m + 1], scale=1.0)
            for m in ms:
                nc.tensor.matmul(p2, lhsT=w2[:, m : m + 1], rhs=hs[m][:, hsl],
                                 start=(m == 0), stop=(m == HC - 1))
        o_sb = outp.tile([1, 512], FP32, tag="o")
        nc.vector.tensor_scalar_add(o_sb, p2, sb["b2"][:1, :1])
        nc.sync.dma_start(d["out"][:, bsl], o_sb)


def _build_f32rh(nc, tc, dram, pools):
    """Hybrid: main term in f32r (11-bit, 1 cycle/row), correction terms in
    bf16.  x@W = xr@wr + xh@wl + xl@wh with xr = f32r(x), xl = bf16(x - xr),
    xh = bf16(x) (same for W).  Error ~5e-7 relative -- fp32-grade -- at the
    same 3-cycles/row PE cost as bf16x3.

    SBUF budget forces: streamed feats tiles, per-n h chunks, bf16 "high"
    planes derived on-device from the f32r planes (zero extra DMA for them).
    """
    big, consts, tmp, outp, pp = pools
    d = dram
    sb = {}
    sb["noff"] = consts.tile([R, NCOL], FP32, tag="noff", name="noff_sb")
    sb["coef"] = consts.tile([R, NCOL], FP32, tag="coef", name="coef_sb")
    sb["b1"] = consts.tile([128, HC], FP32, tag="b1", name="b1_sb")
    sb["b2"] = consts.tile([1, 1], FP32, tag="b2", name="b2_sb")
    sb["ones"] = consts.tile([1, 128], FP32, tag="ones", name="ones_sb")
    nc.vector.memset(sb["ones"], 1.0)
    for key, src in [("noff", d["noff"]), ("coef", d["coef"]),
                     ("b1", d["b1c"]), ("b2", d["b2c"])]:
        nc.sync.dma_start(sb[key], src[:, :])
    w2f_sb = consts.tile([128, HC], FP32, tag="w2f", name="w2f_sb")
    nc.sync.dma_start(w2f_sb, d["w2c"][:, :])

    xr = [big.tile([128, BL], F32R, tag=f"xr{k}", name=f"xr{k}")
          for k in range(KC)]
    xh = [big.tile([128, BL], BF16, tag=f"xh{k}", name=f"xh{k}")
          for k in range(KC)]
    xl = [big.tile([128, BL], BF16, tag=f"xl{k}", name=f"xl{k}")
          for k in range(KC)]
    wr = [big.tile([128, H], F32R, tag=f"wr{k}", name=f"wr{k}")
          for k in range(KC)]
    wh = [big.tile([128, H], BF16, tag=f"wh{k}", name=f"wh{k}")
          for k in range(KC)]
    wl = [big.tile([128, H], BF16, tag=f"wl{k}", name=f"wl{k}")
          for k in range(KC)]

    # RBF features first (small feats DMAs must beat the big DMAs into the
    # queues; the in-order PE's first instruction waits on them)
    for c in range(NCOL):
        for n in range(NT):
            bsl = slice(n * 512, (n + 1) * 512)
            kk = KC_EMB + c
            fe = tmp.tile([1, 512], FP32, tag="fe")
            nc.sync.dma_start(
                fe, d["feats"][:, c * BL + n * 512 : c * BL + (n + 1) * 512])
            bc = pp.tile([128, 512], FP32, tag="ps", name=f"bc_{c}_{n}")
            nc.tensor.matmul(bc, lhsT=sb["ones"][:, :], rhs=fe,
                             start=True, stop=True)
            d2 = tmp.tile([128, 512], FP32, tag="d2")
            nc.scalar.activation(d2, bc, AF.Square,
                                 bias=sb["noff"][:, c : c + 1], scale=1.0)
            rb = tmp.tile([128, 512], FP32, tag="rb")
            nc.scalar.activation(rb, d2, AF.Exp,
                                 scale=sb["coef"][:, c : c + 1])
            nc.vector.tensor_copy(xr[kk][:, bsl], rb)       # round to f32r
            nc.vector.tensor_copy(xh[kk][:, bsl], rb)       # round to bf16
            back = tmp.tile([128, 512], FP32, tag="back")
            nc.vector.tensor_sub(back, rb, xr[kk][:, bsl].bitcast(FP32))
            nc.vector.tensor_copy(xl[kk][:, bsl], back)     # residual -> bf16

    # big DMAs (k-ascending so the first k-sweep streams) + derived bf16
    # "high" planes (DVE casts of the f32r planes; the 2^-12 difference vs
    # bf16(original) only enters the ~2^-13-scale correction terms)
    for k in range(KC):
        ksl = slice(k * 128, (k + 1) * 128)
        nc.sync.dma_start(wr[k][:, :], d["w1r"][ksl, :])
        nc.sync.dma_start(wl[k][:, :], d["w1lb"][ksl, :])
        if k < KC_EMB:
            nc.sync.dma_start(xr[k][:, :], d["ehr"][ksl, :])
            nc.sync.dma_start(xl[k][:, :], d["ehlb"][ksl, :])
            nc.vector.tensor_copy(xh[k][:, :], xr[k].bitcast(FP32))
        nc.vector.tensor_copy(wh[k][:, :], wr[k].bitcast(FP32))

    hs = [big.tile([128, 512], FP32, tag=f"h{m}", name=f"h{m}")
          for m in range(HC)]
    for n in range(NT):
        bsl = slice(n * 512, (n + 1) * 512)
        hsl = slice(0, 512)
        p2 = pp.tile([1, 512], FP32, tag="ps", name=f"p2_{n}")
        for g in range(2):
            ms = range(4 * g, 4 * g + 4)
            ps = {m: pp.tile([128, 512], FP32, tag="ps", name=f"ps_{n}_{g}_{m}")
                  for m in ms}
            for k in range(KC):
                for m in ms:
                    msl = slice(m * 128, (m + 1) * 128)
                    nc.tensor.matmul(ps[m], lhsT=wr[k][:, msl],
                                     rhs=xr[k][:, bsl],
                                     start=(k == 0), stop=False)
                    nc.tensor.matmul(ps[m], lhsT=wh[k][:, msl],
                                     rhs=xl[k][:, bsl],
                                     start=False, stop=False)
                    nc.tensor.matmul(ps[m], lhsT=wl[k][:, msl],
                                     rhs=xh[k][:, bsl],
                                     start=False, stop=(k == KC - 1))
            for m in ms:
                nc.scalar.activation(hs[m][:, hsl], ps[m], AF.Relu,
                                     bias=sb["b1"][:, m : m + 1], scale=1.0)
            for m in ms:
                nc.tensor.matmul(p2, lhsT=w2f_sb[:, m : m + 1],
                                 rhs=hs[m][:, hsl],
                                 start=(m == 0), stop=(m == HC - 1))
        o_sb = outp.tile([1, 512], FP32, tag="o")
        nc.vector.tensor_scalar_add(o_sb, p2, sb["b2"][:1, :1])
        nc.sync.dma_start(d["out"][:, bsl], o_sb)


def _build_fp32(nc, tc, dram, pools):
    big, consts, tmp, outp, pp = pools
    d = dram
    sb = _consts(nc, consts, dict(
        feats=d["feats"], noff=d["noff"], coef=d["coef"],
        b1=d["b1c"], b2=d["b2c"],
    ))
    w2_sb = consts.tile([128, HC], FP32, tag="w2")
    nc.sync.dma_start(w2_sb, d["w2c"][:, :])

    xt = [big.tile([128, BL], FP32, tag=f"xt{k}", name=f"xt{k}")
          for k in range(KC)]
    w1s = [big.tile([128, H], FP32, tag=f"w1_{k}", name=f"w1_{k}")
           for k in range(KC)]
    hs = [big.tile([128, BL], FP32, tag=f"h{m}", name=f"h{m}")
          for m in range(HC)]

    for k in range(KC):
        nc.sync.dma_start(w1s[k][:, :], d["w1"][k * 128 : (k + 1) * 128, :])
        if k < KC_EMB:
            nc.sync.dma_start(xt[k][:, :], d["embT"][k * 128 : (k + 1) * 128, :])

    for c in range(NCOL):
        for n in range(NT):
            bsl = slice(n * 512, (n + 1) * 512)
            bc = _rbf_psum(nc, pp, sb, c, n)
            d2 = tmp.tile([128, 512], FP32, tag="d2")
            nc.scalar.activation(d2, bc, AF.Square,
                                 bias=sb["noff"][:, c : c + 1], scale=1.0)
            nc.scalar.activation(xt[KC_EMB + c][:, bsl], d2, AF.Exp,
                                 scale=sb["coef"][:, c : c + 1])

    for n in range(NT):
        bsl = slice(n * 512, (n + 1) * 512)
        for g in range(2):
            ms = range(4 * g, 4 * g + 4)
            ps = {m: pp.tile([128, 512], FP32, tag="ps", name=f"ps_{n}_{g}_{m}")
                  for m in ms}
            for k in range(KC):
                for m in ms:
                    nc.tensor.matmul(
                        ps[m], lhsT=w1s[k][:, m * 128 : (m + 1) * 128],
                        rhs=xt[k][:, bsl],
                        start=(k == 0), stop=(k == KC - 1),
                    )
            for m in ms:
                nc.scalar.activation(hs[m][:, bsl], ps[m], AF.Relu,
                                     bias=sb["b1"][:, m : m + 1], scale=1.0)

    for n in range(NT):
        bsl = slice(n * 512, (n + 1) * 512)
        p2 = pp.tile([1, 512], FP32, tag="ps", name=f"p2_{n}")
        for m in range(HC):
            nc.tensor.matmul(p2, lhsT=w2_sb[:, m : m + 1], rhs=hs[m][:, bsl],
                             start=(m == 0), stop=(m == HC - 1))
        o_sb = outp.tile([1, 512], FP32, tag="o")
        nc.vector.tensor_scalar_add(o_sb, p2, sb["b2"][:1, :1])
        nc.sync.dma_start(d["out"][:, bsl], o_sb)


def _build_bf16x3(nc, tc, dram, pools):
    big, consts, tmp, outp, pp = pools
    d = dram
    sb = _consts(nc, consts, dict(
        feats=d["feats"], noff=d["noff"], coef=d["coef"],
        b1=d["b1c"], b2=d["b2c"],
    ))
    w2_sb = consts.tile([128, HC], FP32, tag="w2")
    nc.sync.dma_start(w2_sb, d["w2c"][:, :])

    xh = [big.tile([128, BL], BF16, tag=f"xh{k}", name=f"xh{k}")
          for k in range(KC)]
    xl = [big.tile([128, BL], BF16, tag=f"xl{k}", name=f"xl{k}")
          for k in range(KC)]
    wh = [big.tile([128, H], BF16, tag=f"wh{k}", name=f"wh{k}")
          for k in range(KC)]
    wl = [big.tile([128, H], BF16, tag=f"wl{k}", name=f"wl{k}")
          for k in range(KC)]
    hs = [big.tile([128, BL], FP32, tag=f"h{m}", name=f"h{m}")
          for m in range(HC)]

    for k in range(KC):
        ksl = slice(k * 128, (k + 1) * 128)
        nc.sync.dma_start(wh[k][:, :], d["w1h"][ksl, :])
        nc.sync.dma_start(wl[k][:, :], d["w1l"][ksl, :])
        if k < KC_EMB:
            nc.sync.dma_start(xh[k][:, :], d["ehT"][ksl, :])
            nc.sync.dma_start(xl[k][:, :], d["elT"][ksl, :])

    for c in range(NCOL):
        for n in range(NT):
            bsl = slice(n * 512, (n + 1) * 512)
            kk = KC_EMB + c
            bc = _rbf_psum(nc, pp, sb, c, n)
            d2 = tmp.tile([128, 512], FP32, tag="d2")
            nc.scalar.activation(d2, bc, AF.Square,
                                 bias=sb["noff"][:, c : c + 1], scale=1.0)
            rb = tmp.tile([128, 512], FP32, tag="rb")
            nc.scalar.activation(rb, d2, AF.Exp,
                                 scale=sb["coef"][:, c : c + 1])
            nc.vector.tensor_copy(xh[kk][:, bsl], rb)      # round to bf16
            back = tmp.tile([128, 512], FP32, tag="back")
            nc.vector.tensor_copy(back, xh[kk][:, bsl])    # widen
            nc.vector.tensor_sub(back, rb, back)           # residual
            nc.vector.tensor_copy(xl[kk][:, bsl], back)

    for n in range(NT):
        bsl = slice(n * 512, (n + 1) * 512)
        p2 = pp.tile([1, 512], FP32, tag="ps", name=f"p2_{n}")
        for g in range(2):
            ms = range(4 * g, 4 * g + 4)
            ps = {m: pp.tile([128, 512], FP32, tag="ps", name=f"ps_{n}_{g}_{m}")
                  for m in ms}
            for k in range(KC):
                for m in ms:
                    msl = slice(m * 128, (m + 1) * 128)
                    nc.tensor.matmul(ps[m], lhsT=wh[k][:, msl],
                                     rhs=xh[k][:, bsl],
                                     start=(k == 0), stop=False)
                    nc.tensor.matmul(ps[m], lhsT=wh[k][:, msl],
                                     rhs=xl[k][:, bsl],
                                     start=False, stop=False)
                    nc.tensor.matmul(ps[m], lhsT=wl[k][:, msl],
                                     rhs=xh[k][:, bsl],
                                     start=False, stop=(k == KC - 1))
            for m in ms:
                nc.scalar.activation(hs[m][:, bsl], ps[m], AF.Relu,
                                     bias=sb["b1"][:, m : m + 1], scale=1.0)
            for m in ms:
                nc.tensor.matmul(p2, lhsT=w2_sb[:, m : m + 1],
                                 rhs=hs[m][:, bsl],
                                 start=(m == 0), stop=(m == HC - 1))
        o_sb = outp.tile([1, 512], FP32, tag="o")
        nc.vector.tensor_scalar_add(o_sb, p2, sb["b2"][:1, :1])
        nc.sync.dma_start(d["out"][:, bsl], o_sb)



def _build_bf16x3b(nc, tc, dram, pools):
    """bf16x3 + bf16-split second matmul (saves the 4-cycles/row fp32 PE
    cost of the H->1 dot: 16x853ns -> 48x213ns + overlapped DVE splits)."""
    big, consts, tmp, outp, pp = pools
    d = dram
    sb = _consts(nc, consts, dict(
        feats=d["feats"], noff=d["noff"], coef=d["coef"],
        b1=d["b1c"], b2=d["b2c"],
    ))
    w2h_sb = consts.tile([128, HC], BF16, tag="w2h", name="w2h_sb")
    w2l_sb = consts.tile([128, HC], BF16, tag="w2l", name="w2l_sb")
    nc.sync.dma_start(w2h_sb, d["w2h"][:, :])
    nc.sync.dma_start(w2l_sb, d["w2l"][:, :])

    xh = [big.tile([128, BL], BF16, tag=f"xh{k}", name=f"xh{k}")
          for k in range(KC)]
    xl = [big.tile([128, BL], BF16, tag=f"xl{k}", name=f"xl{k}")
          for k in range(KC)]
    wh = [big.tile([128, H], BF16, tag=f"wh{k}", name=f"wh{k}")
          for k in range(KC)]
    wl = [big.tile([128, H], BF16, tag=f"wl{k}", name=f"wl{k}")
          for k in range(KC)]
    hs = [big.tile([128, BL], FP32, tag=f"h{m}", name=f"h{m}")
          for m in range(HC)]
    hh = [big.tile([128, BL], BF16, tag=f"hh{m}", name=f"hh{m}")
          for m in range(HC)]
    hl = [big.tile([128, BL], BF16, tag=f"hl{m}", name=f"hl{m}")
          for m in range(HC)]

    for k in range(KC):
        ksl = slice(k * 128, (k + 1) * 128)
        nc.sync.dma_start(wh[k][:, :], d["w1h"][ksl, :])
        nc.sync.dma_start(wl[k][:, :], d["w1l"][ksl, :])
        if k < KC_EMB:
            nc.sync.dma_start(xh[k][:, :], d["ehT"][ksl, :])
            nc.sync.dma_start(xl[k][:, :], d["elT"][ksl, :])

    for c in range(NCOL):
        for n in range(NT):
            bsl = slice(n * 512, (n + 1) * 512)
            kk = KC_EMB + c
            bc = _rbf_psum(nc, pp, sb, c, n)
            d2 = tmp.tile([128, 512], FP32, tag="d2")
            nc.scalar.activation(d2, bc, AF.Square,
                                 bias=sb["noff"][:, c : c + 1], scale=1.0)
            rb = tmp.tile([128, 512], FP32, tag="rb")
            nc.scalar.activation(rb, d2, AF.Exp,
                                 scale=sb["coef"][:, c : c + 1])
            nc.vector.tensor_copy(xh[kk][:, bsl], rb)
            back = tmp.tile([128, 512], FP32, tag="back")
            nc.vector.tensor_copy(back, xh[kk][:, bsl])
            nc.vector.tensor_sub(back, rb, back)
            nc.vector.tensor_copy(xl[kk][:, bsl], back)

    for n in range(NT):
        bsl = slice(n * 512, (n + 1) * 512)
        p2 = pp.tile([1, 512], FP32, tag="ps", name=f"p2_{n}")
        for g in range(2):
            ms = range(4 * g, 4 * g + 4)
            ps = {m: pp.tile([128, 512], FP32, tag="ps", name=f"ps_{n}_{g}_{m}")
                  for m in ms}
            for k in range(KC):
                for m in ms:
                    msl = slice(m * 128, (m + 1) * 128)
                    nc.tensor.matmul(ps[m], lhsT=wh[k][:, msl],
                                     rhs=xh[k][:, bsl],
                                     start=(k == 0), stop=False)
                    nc.tensor.matmul(ps[m], lhsT=wh[k][:, msl],
                                     rhs=xl[k][:, bsl],
                                     start=False, stop=False)
                    nc.tensor.matmul(ps[m], lhsT=wl[k][:, msl],
                                     rhs=xh[k][:, bsl],
                                     start=False, stop=(k == KC - 1))
            for m in ms:
                nc.scalar.activation(hs[m][:, bsl], ps[m], AF.Relu,
                                     bias=sb["b1"][:, m : m + 1], scale=1.0)
                nc.vector.tensor_copy(hh[m][:, bsl], hs[m][:, bsl])
                back2 = tmp.tile([128, 512], FP32, tag="back2")
                nc.vector.tensor_copy(back2, hh[m][:, bsl])
                nc.vector.tensor_sub(back2, hs[m][:, bsl], back2)
                nc.vector.tensor_copy(hl[m][:, bsl], back2)
            for m in ms:
                mm = slice(m, m + 1)
                nc.tensor.matmul(p2, lhsT=w2h_sb[:, mm], rhs=hh[m][:, bsl],
                                 start=(m == 0), stop=False)
                nc.tensor.matmul(p2, lhsT=w2h_sb[:, mm], rhs=hl[m][:, bsl],
                                 start=False, stop=False)
                nc.tensor.matmul(p2, lhsT=w2l_sb[:, mm], rhs=hh[m][:, bsl],
                                 start=False, stop=(n >= 0 and m == HC - 1))
        o_sb = outp.tile([1, 512], FP32, tag="o")
        nc.vector.tensor_scalar_add(o_sb, p2, sb["b2"][:1, :1])
        nc.sync.dma_start(d["out"][:, bsl], o_sb)


def _build_opt(nc, tc, d, pools):
    """Single-pass bf16 kernel tuned for the timeline cost model.

    vs the 3-pass modes: 1/3 the matmul1 work (bf16 1 cyc/row, ~3e-3 rel
    err vs the 2e-2 gate), ~11 host-packed DMAs instead of 31 (each
    dma_start holds its sequencer ~2.3us), issued from SP+Act+Pool in
    parallel; rbf K-chunks first so the PE starts on on-device data while
    emb/W1 stream in; |w2| folded into W1/b1 on host and H sorted by
    sign(w2) so the H->1 dot collapses to DVE chunk-adds + 3 M=1 matmuls
    (ones / -ones / one mixed-sign column); feats broadcast via fp16 K=1
    matmuls (1 cyc/row vs fp32's 4).
    """
    big, consts, tmp, outp, pp = pools
    mb = d["_mb"]                  # mixed-sign h-chunk index
    P = list(range(mb))            # full +1 chunks
    N = [m for m in range(HC) if m > mb]
    P_EARLY = [m for m in P if m < 4]  # +1 chunks relu'd in n1's g0 phase

    cb = consts.tile([128, 17], FP32, tag="cb", name="cb_sb")
    redw = consts.tile([128, 11], BF16, tag="redw", name="redw_sb")
    # Dummy matmul chain from ~0.95us until just PAST the first
    # broadcast's ready time (~2.9us): the p-state ramp (ANY idle resets
    # it) then runs uninterrupted, so real matmuls hit full clock ~2.7us
    # sooner.  Small warmsrc memset FIRST: it gates the chain's start.
    warmsrc = consts.tile([1, 256], FP16, tag="warmsrc", name="warmsrc")
    nc.vector.memset(warmsrc, 0.0)
    ones16 = consts.tile([1, 128], FP16, tag="ones16", name="ones16_sb")
    nc.vector.memset(ones16, 1.0)
    # Dummy act at t~0: pulls the 1.3us LoadActFuncSet off the critical
    # path (otherwise it hides behind the first rbf act's semaphore wait).
    atl = consts.tile([1, 1], FP32, tag="atl", name="atl_sb")
    nc.vector.memset(atl, 0.0)
    nc.scalar.activation(atl, atl, AF.Derivative_Erf)
    for i in range(8):
        warm = pp.tile([1, 256], FP32, tag="ps", name=f"warm_ps{i}")
        nc.tensor.matmul(warm, lhsT=warmsrc[:, 0:1], rhs=warmsrc[:, :],
                         start=True, stop=True)

    # --- DMA schedule. Transfers SERIALIZE on the DMA engines (~0.39
    # ns/free-byte) in descriptor-ready order, so small/urgent first;
    # issue cost is ~1.2us on SP/Act sequencers but only ~60ns on Pool's.
    # One SBUF tile per DMA: tile-granular dependency tracking otherwise
    # makes every reader wait on ALL of a tile's DMA writers.
    # fen = [feats c0,c1 | noz] in one round-1 DMA (noz gates the very
    # first broadcast; merging avoids a second queue slot + 900ns sem)
    fen = consts.tile([2, 2560], FP16, tag="fen", name="fen_sb")
    fe0 = fen[:, 0:2048]
    noz = fen[:, 2048:2560]
    fe1 = consts.tile([2, 2048], FP16, tag="fe1", name="fe1_sb")
    xr = [big.tile([128, BL], BF16, tag=f"xr{c}", name=f"xr{c}")
          for c in range(NCOL)]
    xea = big.tile([128, 3 * BL], BF16, tag="xea", name="xea")
    xeb = big.tile([128, 3 * BL], BF16, tag="xeb", name="xeb")
    w1k0 = big.tile([128, H], BF16, tag="w1k0", name="w1k0")
    # W1 split by (k-range, m-half): the m0-3 halves are needed ~10us
    # before the m4-7 halves, which relieves the serial-DMA preamble.
    w13a = big.tile([128, 3 * 512], BF16, tag="w13a", name="w13a")
    w49a = big.tile([128, 6 * 512], BF16, tag="w49a", name="w49a")
    w13b = big.tile([128, 3 * 512], BF16, tag="w13b", name="w13b")
    w49b = big.tile([128, 6 * 512], BF16, tag="w49b", name="w49b")

    # DMA: transfers round-robin one-per-queue per round on a SERIAL
    # engine. SP carries the tiny urgent blobs; Pool stages the big pieces
    # in consumption order; Act issues nothing (its seq runs activations).
    # d["w1b"] host layout: [k0 m0-7 | k1-3 m0-3 | k4-9 m0-3 | k1-3 m4-7
    # | k4-9 m4-7], each [128, .] block column-contiguous.
    O1, O2, O3, O4 = H, H + 1536, H + 4608, H + 6144
    # SP and Act queues hold only the tiny round-1/2 blobs; everything
    # else streams on Pool's queue in exact consumption order, so the
    # round-robin scheduler degenerates to the order written here.
    nc.sync.dma_start(fen, d["fe16"][:, 0:2560])
    nc.sync.dma_start(fe1, d["fe16"][:, 2560:4608])
    nc.gpsimd.dma_start(w1k0, d["w1b"][:, 0:O1])
    nc.gpsimd.dma_start(w13a, d["w1b"][:, O1:O2])
    nc.gpsimd.dma_start(xea, d["xe"][:, 0 : 3 * BL])
    nc.gpsimd.dma_start(w49a[:, 0:1536], d["w1b"][:, O2 : O2 + 1536])
    nc.gpsimd.dma_start(xeb, d["xe"][:, 3 * BL : 6 * BL])
    nc.gpsimd.dma_start(w49a[:, 1536:3072], d["w1b"][:, O2 + 1536 : O3])
    nc.gpsimd.dma_start(cb, d["cb"][:, :])
    nc.gpsimd.dma_start(w13b, d["w1b"][:, O3:O4])
    nc.gpsimd.dma_start(w49b, d["w1b"][:, O4 : O4 + 3072])
    nc.gpsimd.dma_start(redw, d["redw"][:, :])

    def wslice(k, m):
        if k == 0:
            return w1k0[:, m * 128 : (m + 1) * 128]
        t = (w13a if m < 4 else w13b) if k < 4 else (w49a if m < 4 else w49b)
        kk = k - 1 if k < 4 else k - 4
        mm = m % 4
        return t[:, kk * 512 + mm * 128 : kk * 512 + (mm + 1) * 128]

    def xslice(k, n):
        b0 = n * 512
        if k < 4:
            return xr[k][:, b0 : b0 + 512]
        if k < 7:
            return xea[:, (k - 4) * BL + b0 : (k - 4) * BL + b0 + 512]
        return xeb[:, (k - 7) * BL + b0 : (k - 7) * BL + b0 + 512]

    def rbf_bc(c, n):
        """(feat_b - off_r) outer difference via a K=2 matmul:
        [ones; -off_c]^T @ [feats; ones] -- no bias tensor needed, so the
        downstream square/exp use only immediates (no cb DMA dependency)."""
        fe = fe0 if c < 2 else fe1
        cc = c % 2
        bc = pp.tile([128, 512], FP32, tag="ps", name=f"bc_{c}_{n}")
        nc.tensor.matmul(bc, lhsT=noz[:, c * 128 : (c + 1) * 128],
                         rhs=fe[:, cc * BL + n * 512 : cc * BL + (n + 1) * 512],
                         start=True, stop=True)
        return bc

    def rbf_act(c, n, bc):
        """exp(coef*d^2) = (sqrt(pi)/2) * erf'(sqrt(-coef)*d): ONE table
        activation per half-chunk (the sqrt(pi)/2 is folded into W1's rbf
        rows on host).  Falls back to Square+Exp if coef >= 0."""
        import math

        cf = float(d["_coefs"][c])
        if cf < 0.0:
            nc.scalar.activation(xr[c][:, n * 512 : (n + 1) * 512], bc,
                                 AF.Derivative_Erf, scale=math.sqrt(-cf))
        else:
            d2 = tmp.tile([128, 512], FP32, tag="d2")
            nc.scalar.activation(d2, bc, AF.Square)
            nc.scalar.activation(xr[c][:, n * 512 : (n + 1) * 512], d2,
                                 AF.Exp, scale=cf)

    def mm1(n, ms, ks, ps=None, opening=True, closing=True):
        if ps is None:
            ps = {m: pp.tile([128, 512], FP32, tag="ps", name=f"ps_{n}_{m}")
                  for m in ms}
        for k in ks:
            for m in ms:
                nc.tensor.matmul(
                    ps[m], lhsT=wslice(k, m), rhs=xslice(k, n),
                    start=(opening and k == ks[0]),
                    stop=(closing and k == ks[-1]),
                )
        return ps

    def relus(n, ps, hs):
        """relu(ps + b1): odd chunks on DVE (fused add+max tensor_scalar)
        so the last group's drain isn't serialized on Act alone."""
        for m in ps:
            if m % 2 == 1:
                nc.vector.tensor_scalar(hs[m], ps[m], cb[:, 8 + m : 9 + m],
                                        0.0, mybir.AluOpType.add,
                                        mybir.AluOpType.max)
            else:
                nc.scalar.activation(hs[m], ps[m], AF.Relu,
                                     bias=cb[:, 8 + m : 9 + m], scale=1.0)

    def hsum_of(hs, chunks, tag, n):
        if len(chunks) == 1:
            return hs[chunks[0]]
        acc = tmp.tile([128, 512], BF16, tag=tag, name=f"{tag}_{n}")
        nc.vector.tensor_tensor(acc, hs[chunks[0]], hs[chunks[1]],
                                mybir.AluOpType.add)
        for m in chunks[2:]:
            nc.vector.tensor_tensor(acc, acc, hs[m], mybir.AluOpType.add)
        return acc

    def emit_out(n, p2):
        o_sb = outp.tile([1, 512], FP32, tag="o")
        nc.scalar.activation(o_sb, p2, AF.Identity, bias=cb[:1, 16:17],
                             scale=1.0)
        # Pool-issued: ~60ns seq + ~1us desc-gen, off the critical Act/SP path
        nc.gpsimd.dma_start(d["out"][:, n * 512 : (n + 1) * 512], o_sb)

    def reduce_h(n, hs):
        """p2[1,512] = sum_m sgn_m * hs[m] via sign-sorted chunks (cheap:
        3 matmuls + DVE adds; used mid-stream where latency is hidden)."""
        p2 = pp.tile([1, 512], FP32, tag="ps", name=f"p2_{n}")
        terms = [(redw[:, 2:3], hs[mb])]
        if P:
            terms.append((redw[:, 0:1], hsum_of(hs, P, "hp", n)))
        if N:
            terms.append((redw[:, 1:2], hsum_of(hs, N, "hn", n)))
        for i, (w, h) in enumerate(terms):
            nc.tensor.matmul(p2, lhsT=w, rhs=h,
                             start=(i == 0), stop=(i == len(terms) - 1))
        emit_out(n, p2)

    def reduce_h_direct(n, hs, hp):
        """p2 for the final tile: full +1 chunks (m<mb, relu'd long ago)
        come pre-summed (hp, DVE adds off the critical path); the late
        chunks get one signed-column matmul each so the last relu feeds a
        matmul directly."""
        p2 = pp.tile([1, 512], FP32, tag="ps", name=f"p2_{n}")
        terms = [(redw[:, 0:1], hp)] if hp is not None else []
        terms += [(redw[:, 3 + m : 4 + m], hs[m])
                  for m in range(HC) if m not in P_EARLY]
        for i, (w, h) in enumerate(terms):
            nc.tensor.matmul(p2, lhsT=w, rhs=h,
                             start=(i == 0), stop=(i == len(terms) - 1))
        emit_out(n, p2)

    hs0 = {m: big.tile([128, 512], BF16, tag=f"h{m}a", name=f"h{m}a")
           for m in range(HC)}
    hs1 = {m: big.tile([128, 512], BF16, tag=f"h{m}b", name=f"h{m}b")
           for m in range(HC)}

    # PE: fe0-dependent broadcasts (c0, c1) first — fe1 lands ~1.3us after
    # fe0 — then c2/c3; Act processes all of n0's square/exp before n1's
    # so n0's rbf chunks land at ~1.2us cadence.
    bcs = {}
    for c, n in [(0, 0), (1, 0), (0, 1), (1, 1), (2, 0), (3, 0)]:
        bcs[(c, n)] = rbf_bc(c, n)
    for c in range(NCOL):
        rbf_act(c, 0, bcs[(c, 0)])

    # n0 g0 over all k (rbf chunks Act-paced, emb chunks DMA-paced),
    # then n0 g1 (all resident), then n1 groups; relus drain each group
    # while the next group's matmuls keep the PE busy.
    ps = mm1(0, range(0, 4), [0], closing=False)
    for c in range(2, NCOL):
        bcs[(c, 1)] = rbf_bc(c, 1)
    mm1(0, range(0, 4), [1, 2, 3, 4, 5, 6, 7, 8, 9], ps=ps, opening=False)
    for c in range(NCOL):
        rbf_act(c, 1, bcs[(c, 1)])
    relus(0, ps, hs0)
    ps = mm1(0, range(4, 8), list(range(KC)))          # n0 g1
    relus(0, ps, hs0)
    ps = mm1(1, range(0, 4), list(range(KC)))          # n1 g0
    relus(1, ps, hs1)
    reduce_h(0, hs0)                                   # n0 output mid-stream
    hp1 = hsum_of(hs1, P_EARLY, "hp", 1) if P_EARLY else None
    ps = mm1(1, range(4, 8), list(range(KC)))          # n1 g1
    relus(1, ps, hs1)
    reduce_h_direct(1, hs1, hp1)


def _build_nc(mode: str, mb: int = 4, npos: int = 4,
              coefs: tuple = (-80.0, -80.0, -80.0, -80.0)) -> bass.Bass:
    # Bacc (not raw Bass): its finalize() runs move_matmul_waits_to_ldweights
    # + generate_event_semaphores, which split semaphore waits that exceed
    # the per-instruction hardware limit (walrus otherwise fails codegen).
    nc = bacc.Bacc()

    if mode == "opt":
        d = {"_mb": mb, "_npos": npos, "_coefs": coefs}
        d["cb"] = nc.dram_tensor("cb", [128, 17], FP32, kind="ExternalInput")
        d["redw"] = nc.dram_tensor("redw", [128, 11], BF16,
                                   kind="ExternalInput")
        d["fe16"] = nc.dram_tensor("fe16", [2, NCOL * BL + 512], FP16,
                                   kind="ExternalInput")
        d["xe"] = nc.dram_tensor("xe", [128, KC_EMB * BL], BF16,
                                 kind="ExternalInput")
        d["w1b"] = nc.dram_tensor("w1b", [128, KC * H], BF16,
                                  kind="ExternalInput")
        d["out"] = nc.dram_tensor("out", [1, BL], FP32, kind="ExternalOutput")
        with tile.TileContext(nc) as tc:
            with (
                tc.tile_pool(name="big", bufs=1) as big,
                tc.tile_pool(name="consts", bufs=1) as consts,
                tc.tile_pool(name="tmp", bufs=3) as tmp,
                tc.tile_pool(name="outp", bufs=2) as outp,
                tc.tile_pool(name="psum", bufs=8, space="PSUM") as pp,
            ):
                _build_opt(nc, tc, d, (big, consts, tmp, outp, pp))
        nc.finalize()
        return nc

    d = {}
    d["feats"] = nc.dram_tensor("feats", [1, NCOL * BL], FP32,
                                kind="ExternalInput")
    d["b1c"] = nc.dram_tensor("b1c", [128, HC], FP32, kind="ExternalInput")
    d["w2c"] = nc.dram_tensor("w2c", [128, HC], FP32, kind="ExternalInput")
    d["b2c"] = nc.dram_tensor("b2c", [1, 1], FP32, kind="ExternalInput")
    d["noff"] = nc.dram_tensor("noff", [R, NCOL], FP32, kind="ExternalInput")
    d["coef"] = nc.dram_tensor("coef", [R, NCOL], FP32, kind="ExternalInput")
    d["out"] = nc.dram_tensor("out", [1, BL], FP32, kind="ExternalOutput")

    if mode == "fp32":
        d["embT"] = nc.dram_tensor("embT", [D, BL], FP32, kind="ExternalInput")
        d["w1"] = nc.dram_tensor("w1", [KTOT, H], FP32, kind="ExternalInput")
    elif mode in ("bf16x3", "bf16x3b"):
        for n2 in ("ehT", "elT"):
            d[n2] = nc.dram_tensor(n2, [D, BL], BF16, kind="ExternalInput")
        for n2 in ("w1h", "w1l"):
            d[n2] = nc.dram_tensor(n2, [KTOT, H], BF16, kind="ExternalInput")
        if mode == "bf16x3b":
            d["w2h"] = nc.dram_tensor("w2h", [128, HC], BF16,
                                      kind="ExternalInput")
            d["w2l"] = nc.dram_tensor("w2l", [128, HC], BF16,
                                      kind="ExternalInput")
    elif mode in ("f32r1", "f32r3"):
        d["ehr"] = nc.dram_tensor("ehr", [D, BL], F32R, kind="ExternalInput")
        d["w1r"] = nc.dram_tensor("w1r", [KTOT, H], F32R, kind="ExternalInput")
        d["w2r"] = nc.dram_tensor("w2r", [128, HC], F32R, kind="ExternalInput")
        if mode == "f32r3":
            d["ehl"] = nc.dram_tensor("ehl", [D, BL], F32R,
                                      kind="ExternalInput")
            d["w1l"] = nc.dram_tensor("w1l", [KTOT, H], F32R,
                                      kind="ExternalInput")
            d["w2l"] = nc.dram_tensor("w2l", [128, HC], F32R,
                                      kind="ExternalInput")
    elif mode == "f32rh":
        d["ehr"] = nc.dram_tensor("ehr", [D, BL], F32R, kind="ExternalInput")
        d["ehlb"] = nc.dram_tensor("ehlb", [D, BL], BF16, kind="ExternalInput")
        d["w1r"] = nc.dram_tensor("w1r", [KTOT, H], F32R, kind="ExternalInput")
        d["w1lb"] = nc.dram_tensor("w1lb", [KTOT, H], BF16,
                                   kind="ExternalInput")
    else:
        raise ValueError(mode)

    with tile.TileContext(nc) as tc:
        with (
            tc.tile_pool(name="big", bufs=1) as big,
            tc.tile_pool(name="consts", bufs=1) as consts,
            tc.tile_pool(name="tmp", bufs=3) as tmp,
            tc.tile_pool(name="outp", bufs=2) as outp,
            tc.tile_pool(name="psum", bufs=8, space="PSUM") as pp,
        ):
            pools = (big, consts, tmp, outp, pp)
            if mode == "fp32":
                _build_fp32(nc, tc, d, pools)
            elif mode == "bf16x3":
                _build_bf16x3(nc, tc, d, pools)
            elif mode == "bf16x3b":
                _build_bf16x3b(nc, tc, d, pools)
            elif mode == "f32rh":
                _build_f32rh(nc, tc, d, pools)
            else:
                _build_f32r(nc, tc, d, pools, three_pass=(mode == "f32r3"))

    # run Bacc's compile pipeline (wait splitting, register allocation);
    # run_bass_via_pjrt serializes nc.m as-is and never finalizes.
    nc.finalize()
    return nc


def _bf16_pair(a: np.ndarray):
    """Split fp32 array into (hi, lo) bf16 arrays with hi+lo ~ a."""
    import ml_dtypes

    hi = a.astype(ml_dtypes.bfloat16)
    lo = (a - hi.astype(np.float32)).astype(ml_dtypes.bfloat16)
    return hi, lo


def _round_f32r(a: np.ndarray) -> np.ndarray:
    """Round fp32 to f32r (11-bit mantissa, round-half-up at bit 12) --
    bit-exact with the hardware's cast (verified against gpsimd cast-DMA)."""
    v = np.ascontiguousarray(a, dtype=np.float32).view(np.uint32)
    r = (((v.astype(np.uint64) + (1 << 11)) >> 12) << 12).astype(np.uint32)
    return r.view(np.float32)


def _f32r_pair(a: np.ndarray):
    hi = _round_f32r(a)
    lo = _round_f32r(a - hi)
    return hi, lo


_NC_CACHE: dict = {}


def _kernel_opt(emb, feats, rbf_offset, rbf_coeff, W1, b1, W2, b2,
                component_idx):
    import ml_dtypes

    w2 = W2[0, :, 0]                                  # [1024]
    sgn = np.where(w2 >= 0.0, 1.0, -1.0).astype(np.float32)
    perm = np.argsort(-sgn, kind="stable")            # +1 h-dims first
    npos = int((sgn > 0).sum())
    mb = min(npos // 128, HC - 1)                     # mixed-sign chunk

    aw2 = np.abs(w2[perm])
    w1p = W1[0][:, perm] * aw2[None, :]               # |w2| folded into W1
    b1p = b1[0][perm] * aw2
    w1k = np.concatenate([w1p[D:], w1p[:D]], axis=0)  # rbf K-chunks first
    # rbf via erf'(s*d) = (2/sqrt(pi))*exp(-(s*d)^2): fold sqrt(pi)/2 into
    # the W1 rows that contract with each derf-computed rbf chunk
    for c in range(NCOL):
        if rbf_coeff[c] < 0.0:
            w1k[c * 128 : (c + 1) * 128, :] *= np.sqrt(np.pi) / 2.0
    wkm = w1k.reshape(KC, 128, H).transpose(1, 0, 2)  # [128, k, H]
    # block layout: k0 full | k1-3 m0-3 | k4-9 m0-3 | k1-3 m4-7 | k4-9 m4-7
    w1b = np.ascontiguousarray(np.concatenate([
        wkm[:, 0, :],
        wkm[:, 1:4, 0:512].reshape(128, -1),
        wkm[:, 4:10, 0:512].reshape(128, -1),
        wkm[:, 1:4, 512:1024].reshape(128, -1),
        wkm[:, 4:10, 512:1024].reshape(128, -1),
    ], axis=1)).astype(ml_dtypes.bfloat16)

    cb = np.zeros((128, 17), np.float32)
    cb[:, 0:4] = -rbf_offset.T
    cb[:, 4:8] = np.broadcast_to(rbf_coeff[None, :], (R, NCOL))
    cb[:, 8:16] = b1p.reshape(HC, 128).T
    cb[:, 16] = float(b2[0, 0])

    redw = np.ones((128, 11), np.float32)
    redw[:, 1] = -1.0
    redw[:, 2] = sgn[perm][mb * 128 : (mb + 1) * 128]
    redw[:, 3:11] = sgn[perm].reshape(HC, 128).T       # per-chunk sign cols
    redw = redw.astype(ml_dtypes.bfloat16)

    noz = np.empty((2, 512), np.float32)
    noz[0] = 1.0
    noz[1] = -rbf_offset.reshape(512)                 # [c*128+r] = -off[c,r]

    shared = dict(cb=cb, redw=redw, w1b=w1b)
    in_maps = []
    for i in range(NCORES):
        s = slice(i * BL, (i + 1) * BL)
        f = feats[:, s]                               # [4, 1024]
        # [feats c0,c1 | noz | feats c2,c3], row 1 = ones except noz rows
        fe16 = np.ones((2, NCOL * BL + 512), np.float32)
        fe16[0, 0:2048] = f[0:2].reshape(-1)
        fe16[:, 2048:2560] = noz
        fe16[0, 2560:4608] = f[2:4].reshape(-1)
        fe16 = fe16.astype(np.float16)
        embT = emb[s].T                               # [768, 1024]
        xe = np.ascontiguousarray(
            embT.reshape(KC_EMB, 128, BL).transpose(1, 0, 2)
            .reshape(128, KC_EMB * BL)
        ).astype(ml_dtypes.bfloat16)
        in_maps.append(dict(fe16=fe16, xe=xe, **shared))

    coefs = tuple(float(c) for c in rbf_coeff)
    key = ("opt", mb, npos, coefs)
    if key not in _NC_CACHE:
        _NC_CACHE[key] = _build_nc("opt", mb=mb, npos=npos, coefs=coefs)

    res = run_bass_kernel_spmd(_NC_CACHE[key], in_maps, list(range(NCORES)))
    pred = np.concatenate(
        [res.results[i]["out"].reshape(BL) for i in range(NCORES)]
    )
    order = np.argsort(component_idx, kind="stable")
    inv = np.argsort(order, kind="stable")
    return pred[inv].reshape(B, 1).astype(np.float32)


def kernel(emb, feats, rbf_offset, rbf_coeff, W1, b1, W2, b2, component_idx):
    mode = os.environ.get("KERNEL_MODE", "opt")
    emb = np.ascontiguousarray(emb, dtype=np.float32)
    feats = np.ascontiguousarray(feats, dtype=np.float32)
    rbf_offset = np.asarray(rbf_offset, dtype=np.float32)
    rbf_coeff = np.asarray(rbf_coeff, dtype=np.float32)
    W1 = np.asarray(W1, dtype=np.float32)
    b1 = np.asarray(b1, dtype=np.float32)
    W2 = np.asarray(W2, dtype=np.float32)
    b2 = np.asarray(b2, dtype=np.float32)
    component_idx = np.asarray(component_idx)

    if mode == "opt":
        return _kernel_opt(emb, feats, rbf_offset, rbf_coeff, W1, b1, W2,
                           b2, component_idx)

    # shared (expert-0 only) tensors
    w1_full = np.ascontiguousarray(W1[0])                        # [1280, 1024]
    w2c = np.ascontiguousarray(W2[0, :, 0].reshape(HC, 128).T)   # [128, 8]
    shared = dict(
        b1c=np.ascontiguousarray(b1[0].reshape(HC, 128).T),      # [128, 8]
        w2c=w2c,
        b2c=b2[0].reshape(1, 1),
        noff=np.ascontiguousarray(-rbf_offset.T),                # [128, 4]
        coef=np.ascontiguousarray(
            np.broadcast_to(rbf_coeff[None, :], (R, NCOL))),     # [128, 4]
    )
    if mode == "fp32":
        shared["w1"] = w1_full
    elif mode in ("bf16x3", "bf16x3b"):
        shared["w1h"], shared["w1l"] = _bf16_pair(w1_full)
        if mode == "bf16x3b":
            shared["w2h"], shared["w2l"] = _bf16_pair(w2c)
    elif mode == "f32rh":
        import ml_dtypes

        shared["w1r"] = _round_f32r(w1_full)
        shared["w1lb"] = (w1_full - shared["w1r"]).astype(ml_dtypes.bfloat16)
    else:
        shared["w1r"], w1l = _f32r_pair(w1_full)
        w2r, w2l = _f32r_pair(w2c)
        shared["w2r"] = w2r
        if mode == "f32r3":
            shared["w1l"] = w1l
            shared["w2l"] = w2l

    in_maps = []
    for i in range(NCORES):
        s = slice(i * BL, (i + 1) * BL)
        m = dict(
            feats=np.ascontiguousarray(feats[:, s]).reshape(1, NCOL * BL),
            **shared,
        )
        embT = np.ascontiguousarray(emb[s].T)                    # [768, 1024]
        if mode == "fp32":
            m["embT"] = embT
        elif mode in ("bf16x3", "bf16x3b"):
            m["ehT"], m["elT"] = _bf16_pair(embT)
        elif mode == "f32rh":
            import ml_dtypes

            m["ehr"] = _round_f32r(embT)
            m["ehlb"] = (embT - m["ehr"]).astype(ml_dtypes.bfloat16)
        else:
            m["ehr"], ehl = _f32r_pair(embT)
            if mode == "f32r3":
                m["ehl"] = ehl
        in_maps.append(m)

    if mode not in _NC_CACHE:
        _NC_CACHE[mode] = _build_nc(mode)

    res = run_bass_kernel_spmd(_NC_CACHE[mode], in_maps, list(range(NCORES)))

    pred = np.concatenate(
        [res.results[i]["out"].reshape(BL) for i in range(NCORES)]
    )                                                            # [8192]

    order = np.argsort(component_idx, kind="stable")
    inv = np.argsort(order, kind="stable")
    return pred[inv].reshape(B, 1).astype(np.float32)

